# revision 40
# baseline (speedup 1.0000x reference)
"""PointNet feature-propagation block on 8 Trainium2 NeuronCores.

Data-parallel over the batch dim: 16 batches -> 2 per core.
Per batch on-device pipeline:
  1. G = feat @ W1^T (+b1) per sparse point (PE); rows [512 x fp16 | xs coords
     as 3 x fp32 | pad] (1280 B) stored to HBM.
  2. Scores S = -d2 via augmented K=5 matmul (PE):
       lhsT rows [2x,2y,2z,|xd|^2,1], rhs rows [x,y,z,-1,-|xs|^2].
  3. Top-8 candidates per dense point: DVE max + max_index; keep 4.
  4. dma_gather of the 4 candidate G-rows (with coords) per dense point.
  5. Exact d2 recomputed from gathered coords; the worst of the 4 candidates
     is excluded by a zero weight; w_m = (1/(sqrt(d2_m)+eps)) normalized.
  6. h = relu(sum_m w_m * g_m)  (fused DVE ops, per-partition scalars).
  7. h^T via XBAR DMA transposes (HWDGE).
  8. out = h @ W2^T (+b2) (PE), stored as (dense, ch) fp32.
"""
import numpy as np

import concourse.bass as bass
import concourse.tile as tile
import concourse.mybir as mybir
from concourse import bacc
from concourse.bass_utils import run_bass_kernel_spmd

B, N1, N2 = 16, 1024, 4096
C_IN, C_OUT = 512, 512
NCAND = 4                    # candidates gathered per dense point
EPS = 1e-10
N_CORES = 8
BPC = B // N_CORES           # batches per core
NCH = N2 // 128              # dense chunks per batch (32)
NR = N1 // 128               # sparse chunks (8)
NQI = C_IN // 128            # input-channel chunks (4)
NQO = C_OUT // 128           # output-channel chunks (4)
JQ = 8                       # dense chunks per gather group
NQUARTER = NCH // JQ         # 4
GROW = 640                   # G row length in fp16 elems (512 feat + 6 coord + pad)

F32 = mybir.dt.float32
F16 = mybir.dt.float16
U16 = mybir.dt.uint16
I16 = mybir.dt.int16
Alu = mybir.AluOpType
Act = mybir.ActivationFunctionType
AxX = mybir.AxisListType.X

# dev-only ablation switches (empty for production)
_ABLATE = set()


def _emit_front(nc, tc, pools, aps, b, include_b1, include_b2):
    (sb, gpool, hpool, htpool, opool, wpool, fpool, wsump, psum_s, psum_g, psum_o) = pools
    (xdT, xsT, featT, xd_pc, xs_pc, w1T_sb, w2T_sb, b1row_sb, b2row_sb,
     ones_sb, ones16_sb, identq_sb, out_ap, g_dram) = aps

    # ---- Stage 1: G rows = [feat @ W1^T (+b1) | xs coords | pad] -----------
    featT_sb = sb.tile([128, NQI, N1], F32, tag="featT")
    for q in range(NQI):
        nc.sync.dma_start(featT_sb[:, q, :], featT[b, q * 128:(q + 1) * 128, :])
    xs_pc_sb = sb.tile([128, NR, 3], F32, tag="xs_pc")
    nc.sync.dma_start(xs_pc_sb[:], xs_pc[b])

    g_sb = sb.tile([128, NR, GROW], F16, tag="g_sb")
    for r in range(NR):
        pg = psum_g.tile([128, C_OUT], F32)
        for q in range(NQI):
            nc.tensor.matmul(
                pg[:], featT_sb[:, q, r * 128:(r + 1) * 128], w1T_sb[:, q, :],
                start=(q == 0), stop=(q == NQI - 1 and not include_b1))
        if include_b1:
            nc.tensor.matmul(pg[:], ones_sb[0:1, :], b1row_sb[:],
                             start=False, stop=True)
        nc.scalar.activation(g_sb[:, r, 0:C_OUT], pg[:], Act.Copy)
    # coords + zero pad
    nc.vector.tensor_copy(g_sb[:, :, C_OUT:C_OUT + 6].bitcast(F32), xs_pc_sb[:])
    nc.vector.memset(g_sb[:, :, C_OUT + 6:GROW], 0.0)
    # store to HBM with row-major (1024, GROW) layout: row r*128+p
    nc.sync.dma_start(g_dram[b].rearrange("(r p) e -> p r e", p=128), g_sb[:])

    # ---- Stage 2: scores + top-8 candidates -------------------------------
    xdT_sb = sb.tile([5, N2], F32, tag="xdT")
    nc.sync.dma_start(xdT_sb[:], xdT[b])
    xsT_sb = sb.tile([5, N1], F32, tag="xsT")
    nc.sync.dma_start(xsT_sb[:], xsT[b])
    xd_pc_sb = fpool.tile([128, NCH, 3], F32, tag="xd_pc")
    nc.sync.dma_start(xd_pc_sb[:], xd_pc[b])

    m8 = fpool.tile([128, NCH, 8], F32, tag="m8")
    i8 = fpool.tile([128, NCH, 8], U16, tag="i8")
    for c in range(NCH):
        ps = psum_s.tile([128, N1], F32)
        lhs = xdT_sb[:, c * 128:(c + 1) * 128]
        nc.tensor.matmul(ps[:, 0:512], lhs, xsT_sb[:, 0:512], start=True, stop=True)
        nc.tensor.matmul(ps[:, 512:1024], lhs, xsT_sb[:, 512:1024], start=True, stop=True)
        if "topk" not in _ABLATE:
            nc.vector.max(m8[:, c, :], ps[:])
            nc.vector.max_index(i8[:, c, :], m8[:, c, :], ps[:])
        else:
            nc.vector.memset(m8[:, c, :], 0.5)
            nc.vector.memset(i8[:, c, :], 1)
    return m8, i8, xd_pc_sb


def _emit_back(nc, tc, pools, aps, b, include_b1, include_b2, front):
    (sb, gpool, hpool, htpool, opool, wpool, fpool, wsump, psum_s, psum_g, psum_o) = pools
    (xdT, xsT, featT, xd_pc, xs_pc, w1T_sb, w2T_sb, b1row_sb, b2row_sb,
     ones_sb, ones16_sb, identq_sb, out_ap, g_dram) = aps
    m8, i8, xd_pc_sb = front

    # ---- Stage 3: candidate index lists in wrapped int16 layout ------------
    idxw = sb.tile([128, NCAND, N2 // 16], I16, tag="idxw")
    if "scatter" in _ABLATE:
        nc.vector.memset(idxw[:], 0)
    for m in range(NCAND) if "scatter" not in _ABLATE else []:
        # dst[p%16, c*8 + p//16] = i8[p, c, m]
        dstv = idxw[0:16, m, :].rearrange("q (c pp) -> q c pp", pp=8)
        for pp in range(8):
            nc.sync.dma_start(dstv[:, :, pp],
                              i8[16 * pp:16 * (pp + 1), :, m].bitcast(I16))
    for r in range(1, 8):
        nc.sync.dma_start(idxw[16 * r:16 * (r + 1), :, :], idxw[0:16, :, :])

    # ---- Stages 4-6: gather, exact d2, weights, weighted sum (per quarter) -
    h_sb = hpool.tile([128, NCH, C_OUT], F16, tag="h_sb")
    if "wsum" in _ABLATE:
        nc.vector.memset(h_sb[:], 1.0)
    for Q in range(NQUARTER):
        js = slice(Q * JQ, (Q + 1) * JQ)
        gk = []
        for m in range(NCAND):
            t = gpool.tile([128, JQ, GROW], F16, tag=f"gk{m}")
            if "gather" in _ABLATE:
                nc.scalar.activation(t[:], t[:], Act.Copy) if False else nc.vector.memset(t[:], 1.0)
            else:
                nc.gpsimd.dma_gather(
                    out_ap=t[:], in_ap=g_dram[b],
                    idxs_ap=idxw[:, m, Q * (JQ * 128 // 16):(Q + 1) * (JQ * 128 // 16)],
                    num_idxs=JQ * 128, num_idxs_reg=JQ * 128, elem_size=GROW)
            gk.append(t)

        # exact d2 for each candidate from gathered coords
        d2q = wpool.tile([128, JQ, NCAND], F32, tag="d2q")
        for m in range(NCAND):
            cview = gk[m][:, :, C_OUT:C_OUT + 6].bitcast(F32)
            diff = wpool.tile([128, JQ, 3], F32, tag="cdiff")
            nc.vector.tensor_tensor(diff[:], xd_pc_sb[:, js, :], cview, Alu.subtract)
            sq = wpool.tile([128, JQ, 3], F32, tag="csq")
            nc.vector.tensor_tensor(sq[:], diff[:], diff[:], Alu.mult)
            nc.vector.tensor_reduce(d2q[:, :, m:m + 1], sq[:], AxX, Alu.add)

        # weights: u = 1/(sqrt(d2)+eps); exclude the max-d2 candidate; norm
        dmax = wpool.tile([128, JQ, 1], F32, tag="dmax")
        nc.vector.tensor_reduce(dmax[:], d2q[:], AxX, Alu.max)
        keep = wpool.tile([128, JQ, NCAND], F32, tag="keep")
        # keep = (d2 < dmax) -> excluded candidate gets 0
        nc.vector.tensor_tensor(keep[:], d2q[:],
                                dmax[:].broadcast_to([128, JQ, NCAND]), Alu.is_lt)
        dist = wpool.tile([128, JQ, NCAND], F32, tag="cdist")
        nc.scalar.activation(dist[:], d2q[:], Act.Sqrt)
        dist2 = wpool.tile([128, JQ, NCAND], F32, tag="cdist2")
        nc.vector.tensor_scalar(dist2[:], dist[:], scalar1=EPS, op0=Alu.add,
                                scalar2=1.0, op1=Alu.mult)
        u = wpool.tile([128, JQ, NCAND], F32, tag="cu")
        nc.vector.reciprocal(u[:], dist2[:])
        u0 = wpool.tile([128, JQ, NCAND], F32, tag="cu0")
        nc.vector.tensor_tensor(u0[:], u[:], keep[:], Alu.mult)
        usum = wpool.tile([128, JQ, 1], F32, tag="cusum")
        nc.vector.tensor_reduce(usum[:], u0[:], AxX, Alu.add)
        sf = wpool.tile([128, JQ, 1], F32, tag="csf")
        nc.vector.reciprocal(sf[:], usum[:])
        w = wpool.tile([128, JQ, NCAND], F32, tag="cw")
        nc.vector.tensor_tensor(w[:], u0[:],
                                sf[:].broadcast_to([128, JQ, NCAND]), Alu.mult)

        for jj in range(JQ) if "wsum" not in _ABLATE else []:
            # t_m = g_m * w_m (tensor_scalar, 4x-capable), then a tree of adds
            tm = []
            for m in range(NCAND):
                t = wsump.tile([128, C_OUT], F16, tag="wsum")
                nc.vector.tensor_scalar(t[:], gk[m][:, jj, 0:C_OUT],
                                        scalar1=w[:, jj, m:m + 1],
                                        op0=Alu.mult, scalar2=1.0, op1=Alu.mult)
                tm.append(t)
            a0 = wsump.tile([128, C_OUT], F16, tag="wsum")
            nc.vector.tensor_tensor(a0[:], tm[0][:], tm[1][:], Alu.add)
            a1 = wsump.tile([128, C_OUT], F16, tag="wsum")
            nc.vector.tensor_tensor(a1[:], tm[2][:], tm[3][:], Alu.add)
            a2 = wsump.tile([128, C_OUT], F16, tag="wsum")
            nc.vector.tensor_tensor(a2[:], a0[:], a1[:], Alu.add)
            nc.vector.tensor_scalar(h_sb[:, Q * JQ + jj, :], a2[:], scalar1=0.0,
                                    op0=Alu.max, scalar2=1.0, op1=Alu.mult)

    # ---- Stage 7: h^T via XBAR DMA transposes (HWDGE) ----------------------
    hT = htpool.tile([128, NQO, N2], F16, tag="hT")
    if "transpose" in _ABLATE:
        nc.vector.memset(hT[:], 1.0)
    else:
        for c in range(NCH):
            nc.sync.dma_start_transpose(hT[:, :, c * 128:(c + 1) * 128],
                                        h_sb[:, c, :])

    # ---- Stage 8: layer 2 --------------------------------------------------
    for c in range(NCH):
        po = psum_o.tile([128, C_OUT], F32)
        for q in range(NQO):
            nc.tensor.matmul(
                po[:], hT[:, q, c * 128:(c + 1) * 128], w2T_sb[:, q, :],
                start=(q == 0), stop=(q == NQO - 1 and not include_b2))
        if include_b2:
            nc.tensor.matmul(po[:], ones16_sb[0:1, :], b2row_sb[:],
                             start=False, stop=True)
        osb = opool.tile([128, C_OUT], F32, tag="osb")
        nc.scalar.activation(osb[:], po[:], Act.Copy)
        nc.sync.dma_start(out_ap[b, c * 128:(c + 1) * 128, :], osb[:])


def _build(include_b1, include_b2):
    nc = bacc.Bacc("TRN2", target_bir_lowering=False, debug=False,
                   num_devices=N_CORES)

    xdT = nc.dram_tensor("xdT", [BPC, 5, N2], F32, kind="ExternalInput").ap()
    xsT = nc.dram_tensor("xsT", [BPC, 5, N1], F32, kind="ExternalInput").ap()
    featT = nc.dram_tensor("featT", [BPC, C_IN, N1], F32, kind="ExternalInput").ap()
    xd_pc = nc.dram_tensor("xd_pc", [BPC, 128, NCH, 3], F32, kind="ExternalInput").ap()
    xs_pc = nc.dram_tensor("xs_pc", [BPC, 128, NR, 3], F32, kind="ExternalInput").ap()
    w1T = nc.dram_tensor("w1T", [C_IN, C_OUT], F32, kind="ExternalInput").ap()
    w2T = nc.dram_tensor("w2T", [C_IN, C_OUT], F16, kind="ExternalInput").ap()
    b1row = nc.dram_tensor("b1row", [1, C_OUT], F32, kind="ExternalInput").ap()
    b2row = nc.dram_tensor("b2row", [1, C_OUT], F16, kind="ExternalInput").ap()
    identq = nc.dram_tensor("identq", [128, N2 // 16], I16, kind="ExternalInput").ap()
    out_ap = nc.dram_tensor("out", [BPC, N2, C_OUT], F32, kind="ExternalOutput").ap()
    g_dram = nc.dram_tensor("gscratch", [BPC, N1, GROW], F16).ap()

    with tile.TileContext(nc) as tc:
        with (
            tc.tile_pool(name="sb", bufs=1) as sb,
            tc.tile_pool(name="gpool", bufs=1) as gpool,
            tc.tile_pool(name="hpool", bufs=1) as hpool,
            tc.tile_pool(name="htpool", bufs=1) as htpool,
            tc.tile_pool(name="opool", bufs=4) as opool,
            tc.tile_pool(name="wpool", bufs=3) as wpool,
            tc.tile_pool(name="fpool", bufs=2) as fpool,
            tc.tile_pool(name="wsump", bufs=12) as wsump,
            tc.tile_pool(name="const", bufs=1) as constp,
            tc.tile_pool(name="psum_s", bufs=3, space="PSUM") as psum_s,
            tc.tile_pool(name="psum_g", bufs=1, space="PSUM") as psum_g,
            tc.tile_pool(name="psum_o", bufs=1, space="PSUM") as psum_o,
        ):
            w1T_sb = constp.tile([128, NQI, C_OUT], F32, tag="w1T")
            for q in range(NQI):
                nc.sync.dma_start(w1T_sb[:, q, :], w1T[q * 128:(q + 1) * 128, :])
            w2T_sb = constp.tile([128, NQI, C_OUT], F16, tag="w2T")
            for q in range(NQI):
                nc.sync.dma_start(w2T_sb[:, q, :], w2T[q * 128:(q + 1) * 128, :])
            b1row_sb = constp.tile([1, C_OUT], F32, tag="b1row")
            b2row_sb = constp.tile([1, C_OUT], F16, tag="b2row")
            identq_sb = constp.tile([128, N2 // 16], I16, tag="identq")
            nc.sync.dma_start(identq_sb[:], identq[:])
            ones_sb = constp.tile([1, 128], F32, tag="ones")
            ones16_sb = constp.tile([1, 128], F16, tag="ones16")
            if include_b1:
                nc.vector.memset(ones_sb[:], 1.0)
                nc.sync.dma_start(b1row_sb[:], b1row[:])
            if include_b2:
                nc.vector.memset(ones16_sb[:], 1.0)
                nc.sync.dma_start(b2row_sb[:], b2row[:])

            pools = (sb, gpool, hpool, htpool, opool, wpool, fpool, wsump, psum_s, psum_g, psum_o)
            aps = (xdT, xsT, featT, xd_pc, xs_pc, w1T_sb, w2T_sb, b1row_sb,
                   b2row_sb, ones_sb, ones16_sb, identq_sb, out_ap, g_dram)
            fronts = []
            for b in range(BPC):
                fronts.append(_emit_front(nc, tc, pools, aps, b,
                                          include_b1, include_b2))
            for b in range(BPC):
                _emit_back(nc, tc, pools, aps, b, include_b1, include_b2,
                           fronts[b])

    nc.compile()
    return nc


_CACHE = {}


def _get_module(include_b1, include_b2):
    key = (include_b1, include_b2, tuple(sorted(_ABLATE)))
    if key not in _CACHE:
        _CACHE[key] = _build(include_b1, include_b2)
    return _CACHE[key]


def make_in_maps(xyz_dense, xyz_sparse, feat_sparse, W1, b1, W2, b2):
    xd = np.asarray(xyz_dense, np.float32)
    xs = np.asarray(xyz_sparse, np.float32)
    feat = np.asarray(feat_sparse, np.float32)

    # augmented score factors: S = 2 xd.xs - |xd|^2 - |xs|^2 = -d2
    xdT = np.empty((B, 5, N2), np.float32)
    xdT[:, 0:3] = 2.0 * xd.transpose(0, 2, 1)
    xdT[:, 3] = np.sum(xd * xd, -1)
    xdT[:, 4] = 1.0
    xsT = np.empty((B, 5, N1), np.float32)
    xsT[:, 0:3] = xs.transpose(0, 2, 1)
    xsT[:, 3] = -1.0
    xsT[:, 4] = -np.sum(xs * xs, -1)

    featT = np.ascontiguousarray(feat.transpose(0, 2, 1))
    # partition-major coords: [p, chunk, 3] with point index = chunk*128 + p
    xd_pc = np.ascontiguousarray(xd.reshape(B, NCH, 128, 3).transpose(0, 2, 1, 3))
    xs_pc = np.ascontiguousarray(xs.reshape(B, NR, 128, 3).transpose(0, 2, 1, 3))
    w1T = np.ascontiguousarray(np.asarray(W1, np.float32).T)
    w2T = np.ascontiguousarray(np.asarray(W2, np.float32).T.astype(np.float16))
    b1row = np.asarray(b1, np.float32).reshape(1, C_OUT)
    ident = np.arange(N2, dtype=np.int16)
    identq = np.zeros((128, N2 // 16), np.int16)
    for r in range(8):
        identq[16 * r:16 * (r + 1), :] = ident.reshape(N2 // 16, 16).T
    b2row = np.asarray(b2, np.float32).astype(np.float16).reshape(1, C_OUT)

    in_maps = []
    for core in range(N_CORES):
        s = slice(core * BPC, (core + 1) * BPC)
        in_maps.append({
            "xdT": np.ascontiguousarray(xdT[s]),
            "xsT": np.ascontiguousarray(xsT[s]),
            "featT": np.ascontiguousarray(featT[s]),
            "xd_pc": np.ascontiguousarray(xd_pc[s]),
            "xs_pc": np.ascontiguousarray(xs_pc[s]),
            "w1T": w1T, "w2T": w2T, "b1row": b1row, "b2row": b2row,
            "identq": identq,
        })
    return in_maps


def kernel(xyz_dense, xyz_sparse, feat_sparse, W1, b1, W2, b2):
    include_b1 = bool(np.any(np.asarray(b1) != 0))
    include_b2 = bool(np.any(np.asarray(b2) != 0))
    nc = _get_module(include_b1, include_b2)
    in_maps = make_in_maps(xyz_dense, xyz_sparse, feat_sparse, W1, b1, W2, b2)
    res = run_bass_kernel_spmd(nc, in_maps, list(range(N_CORES)))
    out = np.concatenate([res.results[i]["out"] for i in range(N_CORES)], axis=0)
    return np.ascontiguousarray(out.astype(np.float32))
